# revision 9
# baseline (speedup 1.0000x reference)
"""Trainium2 Bass kernel for a causal single-head attention layer.

reference:
    v = inp @ Wv + bv; k = inp @ Wk + bk; q = inp @ Wq + bq      # [B,T,H]
    W = softmax(causal_mask(k @ q^T / sqrt(C)))                  # [B,T,T]
    out = W @ v                                                  # [B,T,H]

B=512, T=256, C=384, H=64. Pure data parallel over 8 NeuronCores
(64 batches each); batches are processed in pairs so the projection
matmuls run with a 512-wide moving operand.

Layout: scores are computed transposed (S^T[s,t], s on partitions) with
lhsT=q^T slices / rhs=k^T; exp(S^T) in that layout is directly the
stationary operand for the P@V matmul. Softmax normalization rides a
ones-column appended to V so the P@V matmul also emits row sums; a
reciprocal multiply finishes it. The host pre-transposes inp to
[B/2, C, 2, T] (1KB DMA rows). The V projection is column-packed into
partitions 64..127 of the Q PSUM tile (concurrent col-group matmuls,
one shared PSUM->SBUF copy), then transposed to [s,h] on the PE.
The causal mask is applied by gpsimd affine_select after exp
(max-subtraction is skipped: |scores/sqrt(C)| < ~3 for this problem, so
exp cannot overflow; softmax is shift-invariant). A short warm-up matmul
burst at kernel start brings the PE out of its cold HAM clock state
while the first input DMA is in flight.
"""

import numpy as np
import ml_dtypes

import concourse.bass as bass
import concourse.bacc as bacc
import concourse.mybir as mybir
import concourse.tile as tile
from concourse.bass import broadcast_tensor_aps
from concourse.bass_utils import run_bass_kernel_spmd

N_CORES = 8
B, T, C, H = 512, 256, 384, 64
NB = B // N_CORES          # batches per core
NP = NB // 2               # batch pairs per core
KC = C // 128              # contraction chunks
SCALE = C ** (-0.5)
F32 = mybir.dt.float32
BF16 = mybir.dt.bfloat16
AF = mybir.ActivationFunctionType


def _bmul(nc, out, a, b):
    a2, b2 = broadcast_tensor_aps(a, b)
    nc.vector.tensor_tensor(out, a2, b2, op=mybir.AluOpType.mult)


def _badd(nc, out, a, b):
    a2, b2 = broadcast_tensor_aps(a, b)
    nc.vector.tensor_tensor(out, a2, b2, op=mybir.AluOpType.add)


def build_nc():
    nc = bacc.Bacc("TRN2", target_bir_lowering=False, debug=False)
    x_h = nc.declare_dram_parameter("x", [NP, C, 2, T], BF16, isOutput=False)
    wq_h = nc.declare_dram_parameter("wq", [C, H], BF16, isOutput=False)
    wk_h = nc.declare_dram_parameter("wk", [C, H], BF16, isOutput=False)
    wv_h = nc.declare_dram_parameter("wv", [C, H], BF16, isOutput=False)
    bqv_h = nc.declare_dram_parameter("bqv", [128, 1], F32, isOutput=False)
    bk_h = nc.declare_dram_parameter("bk", [H, 1], F32, isOutput=False)
    bvb_h = nc.declare_dram_parameter("bvb", [128, H], F32, isOutput=False)
    # out[g, u, p, j, h] = attention output for batch 2g+j, t = u*128+p
    out_h = nc.declare_dram_parameter("out", [NP, 2, 128, 2, H], F32, isOutput=True)

    with tile.TileContext(nc) as tc:
        # PE warm-up: ~4.5us of dummy matmuls with no input dependencies so
        # the HAM clock gate reaches 8/8 while the first input DMA streams.
        with (
            tc.tile_pool(name="warm", bufs=1) as warm,
            tc.tile_pool(name="warm_ps", bufs=1, space="PSUM") as warm_ps,
        ):
            wsb = warm.tile([128, 512], BF16, tag="wsb")
            nc.gpsimd.memset(wsb[:], 1.0)
            wps = warm_ps.tile([128, 512], F32, tag="wps")
            for _ in range(10):
                nc.tensor.matmul(wps[:], wsb[:, 0:128], wsb[:], start=True, stop=True)

        with (
            tc.tile_pool(name="const", bufs=1) as const,
            tc.tile_pool(name="xp", bufs=4) as xp,
            tc.tile_pool(name="qkp", bufs=3) as qkp,
            tc.tile_pool(name="exp", bufs=4) as expp,
            tc.tile_pool(name="vp", bufs=3) as vp,
            tc.tile_pool(name="op", bufs=3) as op,
            tc.tile_pool(name="ps_qk", bufs=4, space="PSUM") as ps_qk,
            tc.tile_pool(name="ps_tr", bufs=2, space="PSUM") as ps_tr,
            tc.tile_pool(name="ps_att", bufs=2, space="PSUM") as ps_att,
        ):
            wq_sb = const.tile([128, KC, H], BF16, tag="wq")
            nc.sync.dma_start(wq_sb[:], wq_h.ap().rearrange("(k p) h -> p k h", p=128))
            wk_sb = const.tile([128, KC, H], BF16, tag="wk")
            nc.sync.dma_start(wk_sb[:], wk_h.ap().rearrange("(k p) h -> p k h", p=128))
            wv_sb = const.tile([128, KC, H], BF16, tag="wv")
            nc.sync.dma_start(wv_sb[:], wv_h.ap().rearrange("(k p) h -> p k h", p=128))
            bqv_sb = const.tile([128, 1], F32, tag="bqv")
            nc.sync.dma_start(bqv_sb[:], bqv_h.ap())
            bk_sb = const.tile([H, 1], F32, tag="bk")
            nc.sync.dma_start(bk_sb[:], bk_h.ap())
            bvb_sb = const.tile([128, H], F32, tag="bvb")
            nc.sync.dma_start(bvb_sb[:], bvb_h.ap())

            # identity in partitions 64..127 (for transposing V slices that
            # live in the upper half of the packed q|v tile)
            id_sb = const.tile([128, H], BF16, tag="id")
            nc.gpsimd.memset(id_sb[:], 1.0)
            nc.gpsimd.affine_select(
                out=id_sb[64:128, :], in_=id_sb[64:128, :],
                compare_op=mybir.AluOpType.is_equal, fill=0.0,
                base=0, pattern=[[1, H]], channel_multiplier=-1,
            )

            for g in range(NP):
                # ---- load x^T for the pair: [c_part, k, j, t] -----------
                xt = xp.tile([128, KC, 2, T], BF16, tag="xt", name=f"xt{g}")
                nc.sync.dma_start(
                    xt[:],
                    x_h.ap()[g].rearrange("(k p) j t -> p k j t", p=128),
                )

                # ---- projections: q|v col-packed in one PSUM tile -------
                qv_ps = ps_qk.tile([128, 2, T], F32, tag="qk", name=f"qvps{g}")
                k_ps = ps_qk.tile([H, 2, T], F32, tag="qk", name=f"kps{g}")
                for k in range(KC):
                    nc.tensor.matmul(
                        qv_ps[0:H], wq_sb[:, k, :], xt[:, k],
                        start=(k == 0), stop=(k == KC - 1),
                        skip_group_check=True,
                    )
                    nc.tensor.matmul(
                        qv_ps[H:128], wv_sb[:, k, :], xt[:, k],
                        start=(k == 0), stop=(k == KC - 1),
                        skip_group_check=True,
                    )
                for k in range(KC):
                    nc.tensor.matmul(
                        k_ps[:], wk_sb[:, k, :], xt[:, k],
                        start=(k == 0), stop=(k == KC - 1),
                    )
                qv = qkp.tile([128, 2, T], BF16, tag="qv", name=f"qv{g}")
                nc.scalar.activation(qv[:], qv_ps[:], AF.Identity, bias=bqv_sb[:])
                kt = qkp.tile([H, 2, T], BF16, tag="kt", name=f"kt{g}")
                nc.vector.tensor_scalar_add(kt[:], k_ps[:], bk_sb[:])

                # ---- v^T -> v[s,h] via PE transposes --------------------
                tr_ps = ps_tr.tile([128, 2, 2, H], BF16, tag="tr", name=f"tr{g}")
                for j in range(2):
                    for si in range(2):
                        nc.tensor.transpose(
                            tr_ps[:, j, si, :],
                            qv[H:128, j, si * 128:(si + 1) * 128],
                            id_sb[64:128, :],
                        )
                vo = vp.tile([128, 2, 2, H + 1], BF16, tag="vo", name=f"vo{g}")
                nc.gpsimd.memset(vo[:, :, :, H:H + 1], 1.0)
                _badd(nc, vo[:, :, :, 0:H], tr_ps[:], bvb_sb[:][:, None, None, :])

                # ---- attention (per batch) ------------------------------
                exs = []
                for j in range(2):
                    st_ps = ps_att.tile([128, 384], F32, tag="att", name=f"st{g}_{j}")
                    nc.tensor.matmul(
                        st_ps[:, 0:T], qv[0:H, j, 0:128], kt[:, j, :],
                        start=True, stop=True,
                    )
                    nc.tensor.matmul(
                        st_ps[:, T:T + 128], qv[0:H, j, 128:T], kt[:, j, 128:T],
                        start=True, stop=True,
                    )
                    ex = expp.tile([128, 384], BF16, tag="ex", name=f"ex{g}_{j}")
                    nc.scalar.activation(ex[:], st_ps[:], AF.Exp, scale=SCALE)
                    nc.gpsimd.affine_select(
                        out=ex[:, 0:T], in_=ex[:, 0:T],
                        compare_op=mybir.AluOpType.is_ge, fill=0.0,
                        base=0, pattern=[[1, T]], channel_multiplier=-1,
                    )
                    nc.gpsimd.affine_select(
                        out=ex[:, T:T + 128], in_=ex[:, T:T + 128],
                        compare_op=mybir.AluOpType.is_ge, fill=0.0,
                        base=0, pattern=[[1, 128]], channel_multiplier=-1,
                    )
                    exs.append(ex)

                # ---- out = P @ [v | 1], layout [u, j] for packed store --
                ou_ps = ps_att.tile([128, 2, 2, H + 1], F32, tag="att", name=f"ou{g}")
                for j in range(2):
                    ex = exs[j]
                    nc.tensor.matmul(
                        ou_ps[:, 0, j, :], ex[:, 0:128], vo[:, j, 0, :],
                        start=True, stop=True,
                    )
                    nc.tensor.matmul(
                        ou_ps[:, 1, j, :], ex[:, 128:T], vo[:, j, 0, :],
                        start=True, stop=False,
                    )
                    nc.tensor.matmul(
                        ou_ps[:, 1, j, :], ex[:, T:T + 128], vo[:, j, 1, :],
                        start=False, stop=True,
                    )

                # ---- normalize + store (both batches at once) -----------
                rec = op.tile([128, 2, 2, 1], F32, tag="rec", name=f"rec{g}")
                nc.vector.reciprocal(rec[:], ou_ps[:, :, :, H:H + 1])
                ot = op.tile([128, 2, 2, H], F32, tag="ot", name=f"ot{g}")
                _bmul(nc, ot[:], ou_ps[:, :, :, 0:H], rec[:])
                nc.sync.dma_start(out_h.ap()[g].rearrange("u p j h -> p u j h"), ot[:])
    nc.compile()
    return nc


_NC_CACHE = None


def _get_nc():
    global _NC_CACHE
    if _NC_CACHE is None:
        _NC_CACHE = build_nc()
    return _NC_CACHE


def prep_in_maps(inp, Wv, bv, Wk, bk, Wq, bq):
    """Host-side shard + layout prep. Returns the 8 per-core input maps."""
    bf16 = ml_dtypes.bfloat16
    wq_b = np.ascontiguousarray(np.asarray(Wq, np.float32).astype(bf16))
    wk_b = np.ascontiguousarray(np.asarray(Wk, np.float32).astype(bf16))
    wv_b = np.ascontiguousarray(np.asarray(Wv, np.float32).astype(bf16))
    bqv = np.zeros((128, 1), np.float32)
    bqv[0:H, 0] = np.asarray(bq, np.float32).reshape(H)
    bk_c = np.ascontiguousarray(np.asarray(bk, np.float32).reshape(H, 1))
    bvb = np.ascontiguousarray(
        np.tile(np.asarray(bv, np.float32).reshape(1, H), (128, 1))
    )
    inp = np.asarray(inp, np.float32)
    in_maps = []
    for c in range(N_CORES):
        shard = inp[c * NB:(c + 1) * NB]                    # [NB, T, C]
        # [NP, C, 2, T]: pair-packed, transposed
        x_t = np.ascontiguousarray(
            shard.reshape(NP, 2, T, C).transpose(0, 3, 1, 2).astype(bf16)
        )
        in_maps.append({
            "x": x_t, "wq": wq_b, "wk": wk_b, "wv": wv_b,
            "bqv": bqv, "bk": bk_c, "bvb": bvb,
        })
    return in_maps


def unpack_out(results):
    """results: list of per-core dicts -> full [B, T, H] float32 output."""
    outs = []
    for c in range(N_CORES):
        o = results[c]["out"]                  # [NP, 2, 128, 2, H]
        outs.append(o.transpose(0, 3, 1, 2, 4).reshape(NB, T, H))
    return np.concatenate(outs, axis=0)


def kernel(inp, Wv, bv, Wk, bk, Wq, bq):
    in_maps = prep_in_maps(inp, Wv, bv, Wk, bk, Wq, bq)
    nc = _get_nc()
    res = run_bass_kernel_spmd(nc, in_maps, core_ids=list(range(N_CORES)))
    return unpack_out(res.results)


# revision 10
# speedup vs baseline: 1.0591x; 1.0591x over previous
"""Trainium2 Bass kernel for a causal single-head attention layer.

reference:
    v = inp @ Wv + bv; k = inp @ Wk + bk; q = inp @ Wq + bq      # [B,T,H]
    W = softmax(causal_mask(k @ q^T / sqrt(C)))                  # [B,T,T]
    out = W @ v                                                  # [B,T,H]

B=512, T=256, C=384, H=64. Pure data parallel over 8 NeuronCores
(64 batches each); batches are processed in pairs so the q/k projection
matmuls run with a 512-wide moving operand.

Layout: scores are computed transposed (S^T[s,t], s on partitions) with
lhsT=q^T slices / rhs=k^T; exp(S^T) in that layout is directly the
stationary operand for the P@V matmul, and V is projected directly in
[s,h] layout (x^T chunks stationary), so no transposes anywhere — the
host pre-transposes inp to [B/2, C, 2, T] (1KB DMA rows). Softmax
normalization rides a ones-column appended to V so the P@V matmul also
emits row sums; one reciprocal + one broadcast multiply finish it.
The causal mask is applied by gpsimd affine_select after exp
(max-subtraction is skipped: |scores/sqrt(C)| < ~3 for this problem, so
exp cannot overflow; softmax is shift-invariant). A short warm-up matmul
burst at kernel start brings the PE out of its cold HAM clock state
while the first input DMA is in flight.
"""

import numpy as np
import ml_dtypes

import concourse.bass as bass
import concourse.bacc as bacc
import concourse.mybir as mybir
import concourse.tile as tile
from concourse.bass import broadcast_tensor_aps
from concourse.bass_utils import run_bass_kernel_spmd

N_CORES = 8
B, T, C, H = 512, 256, 384, 64
NB = B // N_CORES          # batches per core
NP = NB // 2               # batch pairs per core
KC = C // 128              # contraction chunks
SCALE = C ** (-0.5)
F32 = mybir.dt.float32
BF16 = mybir.dt.bfloat16
AF = mybir.ActivationFunctionType


def _bmul(nc, out, a, b):
    a2, b2 = broadcast_tensor_aps(a, b)
    nc.vector.tensor_tensor(out, a2, b2, op=mybir.AluOpType.mult)


def _badd(nc, out, a, b):
    a2, b2 = broadcast_tensor_aps(a, b)
    nc.vector.tensor_tensor(out, a2, b2, op=mybir.AluOpType.add)


def build_nc():
    nc = bacc.Bacc("TRN2", target_bir_lowering=False, debug=False)
    x_h = nc.declare_dram_parameter("x", [NP, C, 2, T], BF16, isOutput=False)
    wq_h = nc.declare_dram_parameter("wq", [C, H], BF16, isOutput=False)
    wk_h = nc.declare_dram_parameter("wk", [C, H], BF16, isOutput=False)
    wv_h = nc.declare_dram_parameter("wv", [C, H], BF16, isOutput=False)
    bq_h = nc.declare_dram_parameter("bq", [H, 1], F32, isOutput=False)
    bk_h = nc.declare_dram_parameter("bk", [H, 1], F32, isOutput=False)
    bvb_h = nc.declare_dram_parameter("bvb", [128, H], F32, isOutput=False)
    # out[g, u, p, j, h] = attention output for batch 2g+j, t = u*128+p
    out_h = nc.declare_dram_parameter("out", [NP, 2, 128, 2, H], F32, isOutput=True)

    with tile.TileContext(nc) as tc:
        # PE warm-up: dummy matmuls with no input dependencies so the HAM
        # clock gate reaches 8/8 while the first input DMA streams.
        with (
            tc.tile_pool(name="warm", bufs=1) as warm,
            tc.tile_pool(name="warm_ps", bufs=1, space="PSUM") as warm_ps,
        ):
            wsb = warm.tile([128, 512], BF16, tag="wsb")
            nc.gpsimd.memset(wsb[:], 1.0)
            wps = warm_ps.tile([128, 512], F32, tag="wps")
            for _ in range(10):
                nc.tensor.matmul(wps[:], wsb[:, 0:128], wsb[:], start=True, stop=True)

        with (
            tc.tile_pool(name="const", bufs=1) as const,
            tc.tile_pool(name="xp", bufs=4) as xp,
            tc.tile_pool(name="qkp", bufs=3) as qkp,
            tc.tile_pool(name="exp", bufs=4) as expp,
            tc.tile_pool(name="vp", bufs=3) as vp,
            tc.tile_pool(name="op", bufs=3) as op,
            tc.tile_pool(name="ps_qk", bufs=4, space="PSUM") as ps_qk,
            tc.tile_pool(name="ps_v", bufs=2, space="PSUM") as ps_v,
            tc.tile_pool(name="ps_att", bufs=2, space="PSUM") as ps_att,
        ):
            # first pair's input DMA goes ahead of the constant loads so the
            # projection matmuls can start as soon as the warm-up drains.
            xt0 = xp.tile([128, KC, 2, T], BF16, tag="xt", name="xt0")
            nc.sync.dma_start(
                xt0[:], x_h.ap()[0].rearrange("(k p) j t -> p k j t", p=128)
            )

            wq_sb = const.tile([128, KC, H], BF16, tag="wq")
            nc.sync.dma_start(wq_sb[:], wq_h.ap().rearrange("(k p) h -> p k h", p=128))
            wk_sb = const.tile([128, KC, H], BF16, tag="wk")
            nc.sync.dma_start(wk_sb[:], wk_h.ap().rearrange("(k p) h -> p k h", p=128))
            wv_sb = const.tile([128, KC, H], BF16, tag="wv")
            nc.sync.dma_start(wv_sb[:], wv_h.ap().rearrange("(k p) h -> p k h", p=128))
            bq_sb = const.tile([H, 1], F32, tag="bq")
            nc.sync.dma_start(bq_sb[:], bq_h.ap())
            bk_sb = const.tile([H, 1], F32, tag="bk")
            nc.sync.dma_start(bk_sb[:], bk_h.ap())
            bvb_sb = const.tile([128, H], F32, tag="bvb")
            nc.sync.dma_start(bvb_sb[:], bvb_h.ap())

            for g in range(NP):
                # ---- load x^T for the pair: [c_part, k, j, t] -----------
                if g == 0:
                    xt = xt0
                else:
                    xt = xp.tile([128, KC, 2, T], BF16, tag="xt", name=f"xt{g}")
                    nc.sync.dma_start(
                        xt[:], x_h.ap()[g].rearrange("(k p) j t -> p k j t", p=128)
                    )

                # ---- q^T, k^T projections (both batches at once) --------
                q_ps = ps_qk.tile([H, 2, T], F32, tag="qk", name=f"qps{g}")
                k_ps = ps_qk.tile([H, 2, T], F32, tag="qk", name=f"kps{g}")
                for k in range(KC):
                    nc.tensor.matmul(
                        q_ps[:], wq_sb[:, k, :], xt[:, k],
                        start=(k == 0), stop=(k == KC - 1),
                    )
                for k in range(KC):
                    nc.tensor.matmul(
                        k_ps[:], wk_sb[:, k, :], xt[:, k],
                        start=(k == 0), stop=(k == KC - 1),
                    )
                qt = qkp.tile([H, 2, T], BF16, tag="qt", name=f"qt{g}")
                nc.scalar.activation(qt[:], q_ps[:], AF.Identity, bias=bq_sb[:])
                kt = qkp.tile([H, 2, T], BF16, tag="kt", name=f"kt{g}")
                nc.vector.tensor_scalar_add(kt[:], k_ps[:], bk_sb[:])

                # ---- v in [s, h] layout (x^T chunks stationary) ---------
                v_ps = ps_v.tile([128, 2, 2, H], F32, tag="v", name=f"vps{g}")
                for j in range(2):
                    for si in range(2):
                        for k in range(KC):
                            nc.tensor.matmul(
                                v_ps[:, j, si, :],
                                xt[:, k, j, si * 128:(si + 1) * 128],
                                wv_sb[:, k, :],
                                start=(k == 0), stop=(k == KC - 1),
                            )
                vo = vp.tile([128, 2, 2, H + 1], BF16, tag="vo", name=f"vo{g}")
                nc.gpsimd.memset(vo[:, :, :, H:H + 1], 1.0)
                _badd(nc, vo[:, :, :, 0:H], v_ps[:], bvb_sb[:][:, None, None, :])

                # ---- attention (per batch) ------------------------------
                exs = []
                for j in range(2):
                    st_ps = ps_att.tile([128, 384], F32, tag="att", name=f"st{g}_{j}")
                    nc.tensor.matmul(
                        st_ps[:, 0:T], qt[:, j, 0:128], kt[:, j, :],
                        start=True, stop=True,
                    )
                    nc.tensor.matmul(
                        st_ps[:, T:T + 128], qt[:, j, 128:T], kt[:, j, 128:T],
                        start=True, stop=True,
                    )
                    ex = expp.tile([128, 384], BF16, tag="ex", name=f"ex{g}_{j}")
                    nc.scalar.activation(ex[:], st_ps[:], AF.Exp, scale=SCALE)
                    nc.gpsimd.affine_select(
                        out=ex[:, 0:T], in_=ex[:, 0:T],
                        compare_op=mybir.AluOpType.is_ge, fill=0.0,
                        base=0, pattern=[[1, T]], channel_multiplier=-1,
                    )
                    nc.gpsimd.affine_select(
                        out=ex[:, T:T + 128], in_=ex[:, T:T + 128],
                        compare_op=mybir.AluOpType.is_ge, fill=0.0,
                        base=0, pattern=[[1, 128]], channel_multiplier=-1,
                    )
                    exs.append(ex)

                # ---- out = P @ [v | 1], layout [u, j] for packed store --
                ou_ps = ps_att.tile([128, 2, 2, H + 1], F32, tag="att", name=f"ou{g}")
                for j in range(2):
                    ex = exs[j]
                    nc.tensor.matmul(
                        ou_ps[:, 0, j, :], ex[:, 0:128], vo[:, j, 0, :],
                        start=True, stop=True,
                    )
                    nc.tensor.matmul(
                        ou_ps[:, 1, j, :], ex[:, 128:T], vo[:, j, 0, :],
                        start=True, stop=False,
                    )
                    nc.tensor.matmul(
                        ou_ps[:, 1, j, :], ex[:, T:T + 128], vo[:, j, 1, :],
                        start=False, stop=True,
                    )

                # ---- normalize + store (both batches at once) -----------
                rec = op.tile([128, 2, 2, 1], F32, tag="rec", name=f"rec{g}")
                nc.vector.reciprocal(rec[:], ou_ps[:, :, :, H:H + 1])
                ot = op.tile([128, 2, 2, H], F32, tag="ot", name=f"ot{g}")
                _bmul(nc, ot[:], ou_ps[:, :, :, 0:H], rec[:])
                nc.sync.dma_start(
                    out_h.ap()[g].rearrange("u p j h -> p u j h"), ot[:]
                )
    nc.compile()
    return nc


_NC_CACHE = None


def _get_nc():
    global _NC_CACHE
    if _NC_CACHE is None:
        _NC_CACHE = build_nc()
    return _NC_CACHE


def prep_in_maps(inp, Wv, bv, Wk, bk, Wq, bq):
    """Host-side shard + layout prep. Returns the 8 per-core input maps."""
    bf16 = ml_dtypes.bfloat16
    wq_b = np.ascontiguousarray(np.asarray(Wq, np.float32).astype(bf16))
    wk_b = np.ascontiguousarray(np.asarray(Wk, np.float32).astype(bf16))
    wv_b = np.ascontiguousarray(np.asarray(Wv, np.float32).astype(bf16))
    bq_c = np.ascontiguousarray(np.asarray(bq, np.float32).reshape(H, 1))
    bk_c = np.ascontiguousarray(np.asarray(bk, np.float32).reshape(H, 1))
    bvb = np.ascontiguousarray(
        np.tile(np.asarray(bv, np.float32).reshape(1, H), (128, 1))
    )
    inp = np.asarray(inp, np.float32)
    in_maps = []
    for c in range(N_CORES):
        shard = inp[c * NB:(c + 1) * NB]                    # [NB, T, C]
        x_t = np.ascontiguousarray(
            shard.reshape(NP, 2, T, C).transpose(0, 3, 1, 2).astype(bf16)
        )                                                    # [NP, C, 2, T]
        in_maps.append({
            "x": x_t, "wq": wq_b, "wk": wk_b, "wv": wv_b,
            "bq": bq_c, "bk": bk_c, "bvb": bvb,
        })
    return in_maps


def unpack_out(results):
    """results: list of per-core dicts -> full [B, T, H] float32 output."""
    outs = []
    for c in range(N_CORES):
        o = results[c]["out"]                  # [NP, 2, 128, 2, H]
        outs.append(o.transpose(0, 3, 1, 2, 4).reshape(NB, T, H))
    return np.concatenate(outs, axis=0)


def kernel(inp, Wv, bv, Wk, bk, Wq, bq):
    in_maps = prep_in_maps(inp, Wv, bv, Wk, bk, Wq, bq)
    nc = _get_nc()
    res = run_bass_kernel_spmd(nc, in_maps, core_ids=list(range(N_CORES)))
    return unpack_out(res.results)


# revision 12
# speedup vs baseline: 1.0661x; 1.0066x over previous
"""Trainium2 Bass kernel for a causal single-head attention layer.

reference:
    v = inp @ Wv + bv; k = inp @ Wk + bk; q = inp @ Wq + bq      # [B,T,H]
    W = softmax(causal_mask(k @ q^T / sqrt(C)))                  # [B,T,T]
    out = W @ v                                                  # [B,T,H]

B=512, T=256, C=384, H=64. Pure data parallel over 8 NeuronCores
(64 batches each); batches are processed in pairs so the q/k projection
matmuls run with a 512-wide moving operand.

Layout: scores are computed transposed (S^T[s,t], s on partitions) with
lhsT=q^T slices / rhs=k^T; exp(S^T) in that layout is directly the
stationary operand for the P@V matmul, and V is projected directly in
[s,h] layout (x^T chunks stationary), so no transposes anywhere — the
host pre-transposes inp to [B/2, C, 2, T] (1KB DMA rows). Softmax
normalization rides a ones-column appended to V so the P@V matmul also
emits row sums; one reciprocal + one broadcast multiply finish it.
The causal mask is applied by gpsimd affine_select after exp
(max-subtraction is skipped: |scores/sqrt(C)| < ~3 for this problem, so
exp cannot overflow; softmax is shift-invariant). A short warm-up matmul
burst at kernel start brings the PE out of its cold HAM clock state
while the first input DMA is in flight.
"""

import numpy as np
import ml_dtypes

import concourse.bass as bass
import concourse.bacc as bacc
import concourse.mybir as mybir
import concourse.tile as tile
from concourse.bass import broadcast_tensor_aps
from concourse.bass_utils import run_bass_kernel_spmd

N_CORES = 8
B, T, C, H = 512, 256, 384, 64
NB = B // N_CORES          # batches per core
NP = NB // 2               # batch pairs per core
KC = C // 128              # contraction chunks
SCALE = C ** (-0.5)
F32 = mybir.dt.float32
BF16 = mybir.dt.bfloat16
AF = mybir.ActivationFunctionType


def _bmul(nc, out, a, b):
    a2, b2 = broadcast_tensor_aps(a, b)
    nc.vector.tensor_tensor(out, a2, b2, op=mybir.AluOpType.mult)


def _badd(nc, out, a, b):
    a2, b2 = broadcast_tensor_aps(a, b)
    nc.vector.tensor_tensor(out, a2, b2, op=mybir.AluOpType.add)


def build_nc():
    nc = bacc.Bacc("TRN2", target_bir_lowering=False, debug=False)
    x_h = nc.declare_dram_parameter("x", [NP, C, 2, T], BF16, isOutput=False)
    wq_h = nc.declare_dram_parameter("wq", [C, H], BF16, isOutput=False)
    wk_h = nc.declare_dram_parameter("wk", [C, H], BF16, isOutput=False)
    wv_h = nc.declare_dram_parameter("wv", [C, H], BF16, isOutput=False)
    bq_h = nc.declare_dram_parameter("bq", [H, 1], F32, isOutput=False)
    bk_h = nc.declare_dram_parameter("bk", [H, 1], F32, isOutput=False)
    bvb_h = nc.declare_dram_parameter("bvb", [128, H], F32, isOutput=False)
    # out[g, u, p, j, h] = attention output for batch 2g+j, t = u*128+p
    out_h = nc.declare_dram_parameter("out", [NP, 2, 128, 2, H], F32, isOutput=True)

    with tile.TileContext(nc) as tc:
        # PE warm-up: dummy matmuls with no input dependencies so the HAM
        # clock gate reaches 8/8 while the first input DMA streams. The
        # warm-up SBUF tile lives in the long-lived const pool so the
        # constant DMAs don't inherit an address-reuse dependency on it.
        with (
            tc.tile_pool(name="const", bufs=1) as const,
            tc.tile_pool(name="xp", bufs=4) as xp,
            tc.tile_pool(name="qkp", bufs=3) as qkp,
            tc.tile_pool(name="exp", bufs=4) as expp,
            tc.tile_pool(name="vp", bufs=3) as vp,
            tc.tile_pool(name="op", bufs=3) as op,
        ):
            with tc.tile_pool(name="warm_ps", bufs=1, space="PSUM") as warm_ps:
                wsb = const.tile([128, 512], BF16, tag="wsb")
                nc.gpsimd.memset(wsb[:], 1.0)
                wps = warm_ps.tile([128, 512], F32, tag="wps")
                for _ in range(12):
                    nc.tensor.matmul(
                        wps[:], wsb[:, 0:128], wsb[:], start=True, stop=True
                    )
            ctx_ps = tc.tile_pool(name="ps_qk", bufs=4, space="PSUM")
            ps_qk = ctx_ps.__enter__()
            ctx_v = tc.tile_pool(name="ps_v", bufs=2, space="PSUM")
            ps_v = ctx_v.__enter__()
            ctx_att = tc.tile_pool(name="ps_att", bufs=2, space="PSUM")
            ps_att = ctx_att.__enter__()

            # first pair's input DMA goes ahead of the constant loads so the
            # projection matmuls can start as soon as the warm-up drains.
            xt0 = xp.tile([128, KC, 2, T], BF16, tag="xt", name="xt0")
            nc.sync.dma_start(
                xt0[:], x_h.ap()[0].rearrange("(k p) j t -> p k j t", p=128)
            )

            wq_sb = const.tile([128, KC, H], BF16, tag="wq")
            nc.sync.dma_start(wq_sb[:], wq_h.ap().rearrange("(k p) h -> p k h", p=128))
            wk_sb = const.tile([128, KC, H], BF16, tag="wk")
            nc.sync.dma_start(wk_sb[:], wk_h.ap().rearrange("(k p) h -> p k h", p=128))
            wv_sb = const.tile([128, KC, H], BF16, tag="wv")
            nc.sync.dma_start(wv_sb[:], wv_h.ap().rearrange("(k p) h -> p k h", p=128))
            bq_sb = const.tile([H, 1], F32, tag="bq")
            nc.sync.dma_start(bq_sb[:], bq_h.ap())
            bk_sb = const.tile([H, 1], F32, tag="bk")
            nc.sync.dma_start(bk_sb[:], bk_h.ap())
            bvb_sb = const.tile([128, H], F32, tag="bvb")
            nc.sync.dma_start(bvb_sb[:], bvb_h.ap())

            for g in range(NP):
                # ---- load x^T for the pair: [c_part, k, j, t] -----------
                if g == 0:
                    xt = xt0
                else:
                    xt = xp.tile([128, KC, 2, T], BF16, tag="xt", name=f"xt{g}")
                    nc.sync.dma_start(
                        xt[:], x_h.ap()[g].rearrange("(k p) j t -> p k j t", p=128)
                    )

                # ---- q^T, k^T projections (both batches at once) --------
                q_ps = ps_qk.tile([H, 2, T], F32, tag="qk", name=f"qps{g}")
                k_ps = ps_qk.tile([H, 2, T], F32, tag="qk", name=f"kps{g}")
                for k in range(KC):
                    nc.tensor.matmul(
                        q_ps[:], wq_sb[:, k, :], xt[:, k],
                        start=(k == 0), stop=(k == KC - 1),
                    )
                for k in range(KC):
                    nc.tensor.matmul(
                        k_ps[:], wk_sb[:, k, :], xt[:, k],
                        start=(k == 0), stop=(k == KC - 1),
                    )
                qt = qkp.tile([H, 2, T], BF16, tag="qt", name=f"qt{g}")
                nc.scalar.activation(qt[:], q_ps[:], AF.Identity, bias=bq_sb[:])
                kt = qkp.tile([H, 2, T], BF16, tag="kt", name=f"kt{g}")
                nc.vector.tensor_scalar_add(kt[:], k_ps[:], bk_sb[:])

                # ---- v in [s, h] layout (x^T chunks stationary) ---------
                v_ps = ps_v.tile([128, 2, 2, H], F32, tag="v", name=f"vps{g}")
                for j in range(2):
                    for si in range(2):
                        for k in range(KC):
                            nc.tensor.matmul(
                                v_ps[:, j, si, :],
                                xt[:, k, j, si * 128:(si + 1) * 128],
                                wv_sb[:, k, :],
                                start=(k == 0), stop=(k == KC - 1),
                            )
                vo = vp.tile([128, 2, 2, H + 1], BF16, tag="vo", name=f"vo{g}")
                nc.gpsimd.memset(vo[:, :, :, H:H + 1], 1.0)
                _badd(nc, vo[:, :, :, 0:H], v_ps[:], bvb_sb[:][:, None, None, :])

                # ---- attention (per batch) ------------------------------
                exs = []
                for j in range(2):
                    st_ps = ps_att.tile([128, 384], F32, tag="att", name=f"st{g}_{j}")
                    nc.tensor.matmul(
                        st_ps[:, 0:T], qt[:, j, 0:128], kt[:, j, :],
                        start=True, stop=True,
                    )
                    nc.tensor.matmul(
                        st_ps[:, T:T + 128], qt[:, j, 128:T], kt[:, j, 128:T],
                        start=True, stop=True,
                    )
                    ex = expp.tile([128, 384], BF16, tag="ex", name=f"ex{g}_{j}")
                    nc.scalar.activation(ex[:], st_ps[:], AF.Exp, scale=SCALE)
                    nc.gpsimd.affine_select(
                        out=ex[:, 0:T], in_=ex[:, 0:T],
                        compare_op=mybir.AluOpType.is_ge, fill=0.0,
                        base=0, pattern=[[1, T]], channel_multiplier=-1,
                    )
                    nc.gpsimd.affine_select(
                        out=ex[:, T:T + 128], in_=ex[:, T:T + 128],
                        compare_op=mybir.AluOpType.is_ge, fill=0.0,
                        base=0, pattern=[[1, 128]], channel_multiplier=-1,
                    )
                    exs.append(ex)

                # ---- out = P @ [v | 1], layout [u, j] for packed store --
                ou_ps = ps_att.tile([128, 2, 2, H + 1], F32, tag="att", name=f"ou{g}")
                for j in range(2):
                    ex = exs[j]
                    nc.tensor.matmul(
                        ou_ps[:, 0, j, :], ex[:, 0:128], vo[:, j, 0, :],
                        start=True, stop=True,
                    )
                    nc.tensor.matmul(
                        ou_ps[:, 1, j, :], ex[:, 128:T], vo[:, j, 0, :],
                        start=True, stop=False,
                    )
                    nc.tensor.matmul(
                        ou_ps[:, 1, j, :], ex[:, T:T + 128], vo[:, j, 1, :],
                        start=False, stop=True,
                    )

                # ---- normalize + store (both batches at once) -----------
                rec = op.tile([128, 2, 2, 1], F32, tag="rec", name=f"rec{g}")
                nc.vector.reciprocal(rec[:], ou_ps[:, :, :, H:H + 1])
                ot = op.tile([128, 2, 2, H], F32, tag="ot", name=f"ot{g}")
                _bmul(nc, ot[:], ou_ps[:, :, :, 0:H], rec[:])
                nc.sync.dma_start(
                    out_h.ap()[g].rearrange("u p j h -> p u j h"), ot[:]
                )
            ctx_att.__exit__(None, None, None)
            ctx_v.__exit__(None, None, None)
            ctx_ps.__exit__(None, None, None)
    nc.compile()
    return nc


_NC_CACHE = None


def _get_nc():
    global _NC_CACHE
    if _NC_CACHE is None:
        _NC_CACHE = build_nc()
    return _NC_CACHE


def prep_in_maps(inp, Wv, bv, Wk, bk, Wq, bq):
    """Host-side shard + layout prep. Returns the 8 per-core input maps."""
    bf16 = ml_dtypes.bfloat16
    wq_b = np.ascontiguousarray(np.asarray(Wq, np.float32).astype(bf16))
    wk_b = np.ascontiguousarray(np.asarray(Wk, np.float32).astype(bf16))
    wv_b = np.ascontiguousarray(np.asarray(Wv, np.float32).astype(bf16))
    bq_c = np.ascontiguousarray(np.asarray(bq, np.float32).reshape(H, 1))
    bk_c = np.ascontiguousarray(np.asarray(bk, np.float32).reshape(H, 1))
    bvb = np.ascontiguousarray(
        np.tile(np.asarray(bv, np.float32).reshape(1, H), (128, 1))
    )
    inp = np.asarray(inp, np.float32)
    in_maps = []
    for c in range(N_CORES):
        shard = inp[c * NB:(c + 1) * NB]                    # [NB, T, C]
        x_t = np.ascontiguousarray(
            shard.reshape(NP, 2, T, C).transpose(0, 3, 1, 2).astype(bf16)
        )                                                    # [NP, C, 2, T]
        in_maps.append({
            "x": x_t, "wq": wq_b, "wk": wk_b, "wv": wv_b,
            "bq": bq_c, "bk": bk_c, "bvb": bvb,
        })
    return in_maps


def unpack_out(results):
    """results: list of per-core dicts -> full [B, T, H] float32 output."""
    outs = []
    for c in range(N_CORES):
        o = results[c]["out"]                  # [NP, 2, 128, 2, H]
        outs.append(o.transpose(0, 3, 1, 2, 4).reshape(NB, T, H))
    return np.concatenate(outs, axis=0)


def kernel(inp, Wv, bv, Wk, bk, Wq, bq):
    in_maps = prep_in_maps(inp, Wv, bv, Wk, bk, Wq, bq)
    nc = _get_nc()
    res = run_bass_kernel_spmd(nc, in_maps, core_ids=list(range(N_CORES)))
    return unpack_out(res.results)
